# revision 1
# baseline (speedup 1.0000x reference)
"""DiGCNNet forward on 8 Trainium2 NeuronCores, data-parallel over batch.

Math (per batch b):
  adj = mean_t graph_sigs[b]                  # [30, 30]
  xw  = real[b] @ W                           # [30, 256]
  agg = adj^T @ xw + conv_bias                # [30, 256]
  h   = relu(agg)
  ns  = h @ pool_w + pool_b                   # [30]
  lg  = ns @ head_w^T + head_b                # [7]
  out = softmax(lg)

Device strategy per core (64 batches, processed in 16 groups of 4):
  - T-reduce as a PE matmul: ones^T(1/64) @ G with two batches stacked on the
    128 partitions (K=128), out PSUM [2, 900].
  - adj scatter: PSUM->SBUF copy (ACT) then SBUF->SBUF DMA [1,900] -> [30,30]
    diagonal blocks of a [121, 120] block-diagonal lhsT (row 120 = ones for
    the conv_bias contraction row).
  - xw: real loaded transposed via stride-1-partition DMA ([128(f), 4, 120(n)]),
    4 accumulating matmuls against pre-chunked W -> PSUM [120, 256].
  - agg: one block-diagonal matmul [121,120]^T @ [121,256] -> PSUM [120,256]
    (rhs row 120 = conv_bias).
  - relu on ACT, pool via one tensor_tensor_reduce (mult+add, init=pool_b).
  - head: constant block-diag head_w^T [120, 28] matmul -> logits [28, 1].
  - softmax tail on [28, 16] with 7-block partition sums done via tiny matmuls.
"""

from contextlib import ExitStack

import numpy as np

import concourse.bacc as bacc
import concourse.bass as bass
import concourse.tile as tile
from concourse import mybir
from concourse.bass_utils import run_bass_kernel_spmd

F32 = mybir.dt.float32
F32R = mybir.dt.float32r

B, T, N = 512, 64, 30
F_IN, D, C = 512, 256, 7
NCORES = 8
BL = B // NCORES        # 64 batches per core
GPB = 4                 # batches per group
NG = BL // GPB          # 16 groups
NN = N * N              # 900
NB = GPB * N            # 120 stacked node rows per group


def _build_nc():
    nc = bacc.Bacc(None, target_bir_lowering=False)

    gs = nc.dram_tensor("gs", (BL, T, N, N), F32, kind="ExternalInput")
    # real pre-transposed on host to [F_IN, BL*N] so chunk loads are
    # contiguous-innermost for the DMA engines.
    realt = nc.dram_tensor("realt", (F_IN, BL * N), F32, kind="ExternalInput")
    wt = nc.dram_tensor("wt", (128, 4, D), F32, kind="ExternalInput")
    cb = nc.dram_tensor("cb", (1, D), F32, kind="ExternalInput")
    pwb = nc.dram_tensor("pwb", (NB, D), F32, kind="ExternalInput")
    hwblk = nc.dram_tensor("hwblk", (NB, GPB * C), F32, kind="ExternalInput")
    hbb = nc.dram_tensor("hbb", (GPB * C, 1), F32, kind="ExternalInput")
    ones2 = nc.dram_tensor("ones2", (128, 2), F32, kind="ExternalInput")
    ones1 = nc.dram_tensor("ones1", (1, NB), F32, kind="ExternalInput")
    b7 = nc.dram_tensor("b7", (GPB * C, GPB), F32, kind="ExternalInput")
    b7t = nc.dram_tensor("b7t", (GPB, GPB * C), F32, kind="ExternalInput")
    out = nc.dram_tensor("out", (BL, C), F32, kind="ExternalOutput")

    with tile.TileContext(nc) as tc, ExitStack() as ctx:
        consts = ctx.enter_context(tc.tile_pool(name="consts", bufs=1))
        gt_pool = ctx.enter_context(tc.tile_pool(name="gt", bufs=8))
        adjs_pool = ctx.enter_context(tc.tile_pool(name="adjs", bufs=6))
        adjb_pool = ctx.enter_context(tc.tile_pool(name="adjb", bufs=16))
        xwb_pool = ctx.enter_context(tc.tile_pool(name="xwb", bufs=2))
        h_pool = ctx.enter_context(tc.tile_pool(name="h", bufs=2))
        scr_pool = ctx.enter_context(tc.tile_pool(name="scr", bufs=2))
        ns_pool = ctx.enter_context(tc.tile_pool(name="ns", bufs=2))
        tail_pool = ctx.enter_context(tc.tile_pool(name="tail", bufs=1))
        adjp_pool = ctx.enter_context(
            tc.tile_pool(name="adjp", bufs=2, space=bass.MemorySpace.PSUM)
        )
        xwp_pool = ctx.enter_context(
            tc.tile_pool(name="xwp", bufs=2, space=bass.MemorySpace.PSUM)
        )
        aggp_pool = ctx.enter_context(
            tc.tile_pool(name="aggp", bufs=1, space=bass.MemorySpace.PSUM)
        )
        smallp_pool = ctx.enter_context(
            tc.tile_pool(name="smallp", bufs=1, space=bass.MemorySpace.PSUM)
        )

        def load_const(dram, shape, dtype=F32):
            t = consts.tile(shape, dtype, tag=dram.name)
            src_ap = dram[:].bitcast(dtype) if dtype is not F32 else dram[:]
            nc.scalar.dma_start(t[:], src_ap)
            return t

        wt_sb = load_const(wt, [128, 4, D], F32R)
        cb_sb = load_const(cb, [1, D], F32R)
        pwb_sb = load_const(pwb, [NB, D])
        hw_sb = load_const(hwblk, [NB, GPB * C])
        hbb_sb = load_const(hbb, [GPB * C, 1])
        ones2_sb = load_const(ones2, [128, 2], F32R)
        ones1_sb = load_const(ones1, [1, NB], F32R)
        b7_sb = load_const(b7, [GPB * C, GPB])
        b7t_sb = load_const(b7t, [GPB, GPB * C])

        logits_all = consts.tile([GPB * C, NG], F32, tag="logits_all")

        # whole realt resident in SBUF: [128(f%128), 4(f//128), 1920(b*n)]
        rt_all = consts.tile([128, 4, BL * N], F32R, tag="rt_all")
        nc.sync.dma_start(
            rt_all[:], realt.rearrange("(c p) m -> p c m", p=128).bitcast(F32R)
        )

        # ---- phase A: T-reduce all groups into persistent block-diag tiles
        adjb_tiles = []
        for g in range(NG):
            adjb_t = adjb_pool.tile([NB, NB], F32R, tag="adjb")
            nc.vector.memset(adjb_t[:].bitcast(F32), 0.0)
            adjb_tiles.append(adjb_t)

        for g in range(NG):
            b0 = g * GPB
            adjs_tiles = []
            for p2 in range(2):
                bb = b0 + 2 * p2
                gtile = gt_pool.tile([128, NN], F32R, tag="gt")
                nc.sync.dma_start(
                    gtile[:],
                    gs[bb : bb + 2].rearrange("b t i j -> (b t) (i j)").bitcast(F32R),
                )
                adjp_t = adjp_pool.tile([2, NN], F32, tag="adjp")
                nc.tensor.matmul(
                    adjp_t[:, 0:512], ones2_sb[:], gtile[:, 0:512],
                    start=True, stop=True,
                )
                nc.tensor.matmul(
                    adjp_t[:, 512:NN], ones2_sb[:], gtile[:, 512:NN],
                    start=True, stop=True,
                )
                adjs_t = adjs_pool.tile([2, NN], F32, tag="adjs")
                if p2 == 0:
                    nc.scalar.copy(adjs_t[:], adjp_t[:])
                else:
                    nc.vector.tensor_copy(adjs_t[:], adjp_t[:])
                adjs_tiles.append(adjs_t)
            for k in range(GPB):
                nc.gpsimd.dma_start(
                    adjb_tiles[g][k * N : (k + 1) * N, k * N : (k + 1) * N],
                    adjs_tiles[k // 2][k % 2 : k % 2 + 1, :].bitcast(F32R),
                )

        # ---- phase B: xw -> agg -> relu -> pool -> head per group
        for g in range(NG):
            b0 = g * GPB
            xwp_t = xwp_pool.tile([NB, D], F32, tag="xwp")
            for c4 in range(4):
                nc.tensor.matmul(
                    xwp_t[:], rt_all[:, c4, b0 * N : (b0 + GPB) * N],
                    wt_sb[:, c4, :], start=(c4 == 0), stop=(c4 == 3),
                )
            xwb_t = xwb_pool.tile([NB, D], F32R, tag="xwb")
            nc.vector.tensor_copy(xwb_t[:], xwp_t[:])

            aggp_t = aggp_pool.tile([NB, D], F32, tag="aggp")
            nc.tensor.matmul(
                aggp_t[:], adjb_tiles[g][:], xwb_t[:], start=True, stop=False,
            )
            nc.tensor.matmul(
                aggp_t[:], ones1_sb[:], cb_sb[:], start=False, stop=True,
            )

            h_t = h_pool.tile([NB, D], F32, tag="h")
            nc.scalar.activation(h_t[:], aggp_t[:], mybir.ActivationFunctionType.Relu)
            scr_t = scr_pool.tile([NB, D], F32, tag="scr")
            ns_t = ns_pool.tile([NB, 1], F32, tag="ns")
            nc.vector.tensor_mul(scr_t[:], h_t[:], pwb_sb[:])
            nc.vector.reduce_sum(ns_t[:], scr_t[:], axis=mybir.AxisListType.X)

            lg_t = smallp_pool.tile([GPB * C, 1], F32, tag="small")
            nc.tensor.matmul(lg_t[:], hw_sb[:], ns_t[:], start=True, stop=True)
            nc.vector.tensor_add(logits_all[:, g : g + 1], lg_t[:], hbb_sb[:])

        # ---- softmax over the 7 classes (partition sub-blocks of 7)
        e_t = tail_pool.tile([GPB * C, NG], F32, tag="e")
        nc.scalar.activation(e_t[:], logits_all[:], mybir.ActivationFunctionType.Exp)
        sum_p = smallp_pool.tile([GPB, NG], F32, tag="small")
        nc.tensor.matmul(sum_p[:], b7_sb[:], e_t[:], start=True, stop=True)
        ssb_t = tail_pool.tile([GPB, NG], F32, tag="ssb")
        nc.vector.tensor_copy(ssb_t[:], sum_p[:])
        bcast_p = smallp_pool.tile([GPB * C, NG], F32, tag="small")
        nc.tensor.matmul(bcast_p[:], b7t_sb[:], ssb_t[:], start=True, stop=True)
        rs_t = tail_pool.tile([GPB * C, NG], F32, tag="rs")
        nc.vector.reciprocal(rs_t[:], bcast_p[:])
        res_t = tail_pool.tile([GPB * C, NG], F32, tag="res")
        nc.vector.tensor_mul(res_t[:], e_t[:], rs_t[:])
        nc.scalar.dma_start(out.rearrange("(g bi) c -> (bi c) g", bi=GPB), res_t[:])

    nc.compile()
    return nc


_NC_CACHE = None


def _get_nc():
    global _NC_CACHE
    if _NC_CACHE is None:
        _NC_CACHE = _build_nc()
    return _NC_CACHE


def _f32c(x):
    return np.ascontiguousarray(np.asarray(x, dtype=np.float32))


def _prepare_in_maps(real, graph_sigs, W, conv_bias, pool_w, pool_b, head_w, head_b):
    real = _f32c(real)
    graph_sigs = _f32c(graph_sigs)
    W = _f32c(W)

    wt = np.ascontiguousarray(
        _f32c(W).reshape(4, 128, D).transpose(1, 0, 2)
    )  # [128(f%128), 4(f//128), 256]
    cb = _f32c(conv_bias).reshape(1, D)
    pwb = np.ascontiguousarray(np.broadcast_to(_f32c(pool_w), (NB, D)))
    hw_t = _f32c(head_w).T  # [30, 7]
    hwblk = np.zeros((NB, GPB * C), dtype=np.float32)
    for k in range(GPB):
        hwblk[k * N : (k + 1) * N, k * C : (k + 1) * C] = hw_t
    # pool_b shifts every node score by a constant; fold it into the head
    # bias: logits[c] += pool_b * sum_j head_w[c, j]
    hb_eff = _f32c(head_b) + np.float32(np.asarray(pool_b)) * _f32c(head_w).sum(axis=1)
    hbb = np.tile(hb_eff, GPB).reshape(GPB * C, 1)
    ones2 = np.zeros((128, 2), dtype=np.float32)
    ones2[0:64, 0] = 1.0 / T
    ones2[64:128, 1] = 1.0 / T
    b7 = np.zeros((GPB * C, GPB), dtype=np.float32)
    for k in range(GPB):
        b7[k * C : (k + 1) * C, k] = 1.0
    b7t = np.ascontiguousarray(b7.T)
    ones1 = np.ones((1, NB), dtype=np.float32)

    consts = {
        "wt": wt, "cb": cb, "pwb": pwb, "hwblk": hwblk,
        "hbb": hbb, "ones2": ones2, "ones1": ones1, "b7": b7, "b7t": b7t,
    }
    in_maps = []
    for c in range(NCORES):
        s = slice(c * BL, (c + 1) * BL)
        in_maps.append(
            {
                "gs": np.ascontiguousarray(graph_sigs[s]),
                "realt": np.ascontiguousarray(
                    real[s].transpose(2, 0, 1).reshape(F_IN, BL * N)
                ),
                **consts,
            }
        )
    return in_maps


def kernel(real, imag, graph_sigs, W, conv_bias, pool_w, pool_b, head_w, head_b):
    del imag  # unused by the forward pass
    in_maps = _prepare_in_maps(
        real, graph_sigs, W, conv_bias, pool_w, pool_b, head_w, head_b
    )
    nc = _get_nc()
    res = run_bass_kernel_spmd(nc, in_maps, core_ids=list(range(NCORES)))
    return np.concatenate([res.results[c]["out"] for c in range(NCORES)], axis=0)



# revision 15
# speedup vs baseline: 1.2226x; 1.2226x over previous
"""DiGCNNet forward on 8 Trainium2 NeuronCores, data-parallel over batch.

Math (per batch b):
  adj = mean_t graph_sigs[b]                  # [30, 30]
  xw  = real[b] @ W                           # [30, 256]
  agg = adj^T @ xw + conv_bias                # [30, 256]
  h   = relu(agg)
  ns  = h @ pool_w + pool_b                   # [30]
  lg  = ns @ head_w^T + head_b                # [7]
  out = softmax(lg)

Device strategy per core (64 batches, 16 groups of 4, bf16 inputs):
  - gs cast to bf16 on host, stored [4096, 900] (rows = (b, t)).  Loaded in 8
    chunks of 512 rows with partition p <- row 4p+c ("(p c) m"), giving 7.2KB
    contiguous DMA descriptors.  Batch b of a chunk occupies partitions
    [16b, 16b+16) for every c, so a block-structured ones8 [128, 8] matmul
    (accumulated over c=0..3) T-reduces 8 batches -> PSUM [8, 900].
  - adj -> block-diagonal via a DRAM round-trip (SBUF-side DMA APs cannot
    split a free dim across partitions, but DRAM APs have arbitrary strides):
    per group scatter-write adjs[4h:4h+4] into a zero-initialized DRAM
    scratch laid out [128 rows (32k+i), 16 groups, 120 cols (30k+j)], then
    read back per chunk as one [128, 2, 120] tile.  Pad rows stay zero.
  - xw: realT padded to [512, 2048] bf16 on host (group g at cols 128g+32k+n,
    zeros at n=30,31); 4 accumulating matmuls vs W chunks -> PSUM [128, 256].
  - agg: bias matmul (ones x conv_bias) + ONE block-diag matmul
    bd[128, 120]^T @ xwb[128, 256] -> PSUM [120, 256] (rows 30k+j).
  - relu + pool-weight mult + free-dim sum fused in one DVE
    scalar_tensor_tensor (accum_out) -> node scores ns [120, 1].
  - head: [120, 28] matmul -> logits accumulated directly in a PSUM [28, 16]
    tile (one column per group); pool_b/head_b folded into the softmax exp
    bias; softmax tail as 7-block partition sums via tiny matmuls.
"""

from contextlib import ExitStack

import ml_dtypes
import numpy as np

import concourse.bacc as bacc
import concourse.bass as bass
import concourse.tile as tile
from concourse import mybir
from concourse.bass_utils import run_bass_kernel_spmd

F32 = mybir.dt.float32
BF16 = mybir.dt.bfloat16
NP_BF16 = ml_dtypes.bfloat16

B, T, N = 512, 64, 30
F_IN, D, C = 512, 256, 7
NCORES = 8
BL = B // NCORES        # 64 batches per core
GPB = 4                 # batches per group
NG = BL // GPB          # 16 groups
NN = N * N              # 900
NB = GPB * N            # 120 packed rows per group
PB = 32                 # padded rows per batch block
NCHUNK = 8              # gs chunks; each = 512 rows = 8 batches = 2 groups
BPC = 8                 # batches per chunk
BDW = NG * NB           # 1920: bd scratch row width


def _build_nc():
    nc = bacc.Bacc(None, target_bir_lowering=False)

    gs8 = nc.dram_tensor("gs8", (BL * T, NN), BF16, kind="ExternalInput")
    rtp = nc.dram_tensor("rtp", (F_IN, NG * 128), BF16, kind="ExternalInput")
    wt = nc.dram_tensor("wt", (128, 4, D), BF16, kind="ExternalInput")
    # consolidated constants: one bf16 blob, one f32 blob (single DMA each)
    # cbh[:, 0:8] = ones8; cbh[0, 8:136] = ones1; cbh[0, 136:392] = conv_bias
    cbh = nc.dram_tensor("cbh", (128, 392), BF16, kind="ExternalInput")
    # cf[:, 0:256] = pwb; cf[0:120, 256:284] = hwblk; cf[0:28, 284] = hbb;
    # cf[0:28, 285:289] = b7; cf[0:4, 289:317] = b7t
    cf = nc.dram_tensor("cf", (128, 320), F32, kind="ExternalInput")
    # block-diag scratch: element (32k+i)*1920 + g*120 + 30k+j
    bdram = nc.dram_tensor("bdram", (128, BDW), BF16, kind="Internal")
    out = nc.dram_tensor("out", (BL, C), F32, kind="ExternalOutput")

    with tile.TileContext(nc) as tc, ExitStack() as ctx:
        consts = ctx.enter_context(tc.tile_pool(name="consts", bufs=1))
        gs_pool = ctx.enter_context(tc.tile_pool(name="gsp", bufs=NCHUNK))
        adjs_pool = ctx.enter_context(tc.tile_pool(name="adjs", bufs=4))
        bd_pool = ctx.enter_context(tc.tile_pool(name="bd", bufs=3))
        xwb_pool = ctx.enter_context(tc.tile_pool(name="xwb", bufs=2))
        scr_pool = ctx.enter_context(tc.tile_pool(name="scr", bufs=2))
        ns_pool = ctx.enter_context(tc.tile_pool(name="ns", bufs=2))
        tail_pool = ctx.enter_context(tc.tile_pool(name="tail", bufs=1))
        adjp_pool = ctx.enter_context(
            tc.tile_pool(name="adjp", bufs=2, space=bass.MemorySpace.PSUM)
        )
        xwp_pool = ctx.enter_context(
            tc.tile_pool(name="xwp", bufs=2, space=bass.MemorySpace.PSUM)
        )
        aggp_pool = ctx.enter_context(
            tc.tile_pool(name="aggp", bufs=1, space=bass.MemorySpace.PSUM)
        )
        lgp_pool = ctx.enter_context(
            tc.tile_pool(name="lgp", bufs=1, space=bass.MemorySpace.PSUM)
        )

        # consts + weights on the scalar queue (gs stream owns sync)
        cbh_sb = consts.tile([128, 392], BF16, tag="cbh")
        nc.scalar.dma_start(cbh_sb[:], cbh[:])
        cf_sb = consts.tile([128, 320], F32, tag="cf")
        nc.scalar.dma_start(cf_sb[:], cf[:])
        ones8_sb = cbh_sb[:, 0:BPC]
        ones1_sb = cbh_sb[0:1, BPC : BPC + NB]
        cb_sb = cbh_sb[0:1, 136 : 136 + D]
        pwb_sb = cf_sb[0:NB, 0:D]
        hw_sb = cf_sb[0:NB, D : D + GPB * C]
        hbb_sb = cf_sb[0 : GPB * C, 284:285]
        b7_sb = cf_sb[0 : GPB * C, 285:289]
        b7t_sb = cf_sb[0:GPB, 289:317]

        wt_sb = consts.tile([128, 4, D], BF16, tag="wt")
        nc.scalar.dma_start(wt_sb[:], wt[:])
        rt_all = consts.tile([128, 4, NG * 128], BF16, tag="rt_all")
        for h4 in range(4):
            cs = h4 * 512
            nc.scalar.dma_start(
                rt_all[:, :, cs : cs + 512],
                rtp[:, cs : cs + 512].rearrange("(p c) m -> p c m", c=4),
            )

        # zero-init the bd scratch (one contiguous write)
        zt = consts.tile([128, BDW], BF16, tag="zt")
        nc.vector.memset(zt[:], 0.0)
        nc.scalar.dma_start(bdram[:], zt[:])

        # gs stream: 8 chunks of [128, 4, 900] on the sync queue
        gs_tiles = []
        for s in range(NCHUNK):
            gt = gs_pool.tile([128, 4, NN], BF16, tag="gt", name=f"gt{s}")
            nc.sync.dma_start(
                gt[:],
                gs8[512 * s : 512 * (s + 1)].rearrange("(p c) m -> p c m", c=4),
            )
            gs_tiles.append(gt)

        logits_p = lgp_pool.tile([GPB * C, NG], F32, tag="logits")
        bd_tiles = {}

        def emit_treduce(s):
            adjp_t = adjp_pool.tile([BPC, NN], F32, tag="adjp")
            for c4 in range(4):
                nc.tensor.matmul(
                    adjp_t[:, 0:512], ones8_sb[:], gs_tiles[s][:, c4, 0:512],
                    start=(c4 == 0), stop=(c4 == 3),
                )
                nc.tensor.matmul(
                    adjp_t[:, 512:NN], ones8_sb[:], gs_tiles[s][:, c4, 512:NN],
                    start=(c4 == 0), stop=(c4 == 3),
                )
            adjs_t = adjs_pool.tile([BPC, NN], BF16, tag="adjs")
            nc.vector.tensor_copy(adjs_t[:, 0:450], adjp_t[:, 0:450])
            nc.scalar.copy(adjs_t[:, 450:NN], adjp_t[:, 450:NN])
            # scatter-write the two groups' diag blocks into the DRAM scratch
            for half, eng in ((0, nc.scalar), (1, nc.gpsimd)):
                g = 2 * s + half
                wdst = bass.AP(
                    bdram, NB * g, [[PB * BDW + N, GPB], [BDW, N], [1, N]]
                )
                eng.dma_start(wdst, adjs_t[4 * half : 4 * half + 4, :])
            # read back both groups as one [128, 2, 120] block-diag tile
            bd_t = bd_pool.tile([128, 2, NB], BF16, tag="bd", name=f"bd{s}")
            nc.gpsimd.dma_start(
                bd_t[:],
                bdram[:, 2 * NB * s : 2 * NB * (s + 1)].rearrange(
                    "p (h m) -> p h m", h=2
                ),
            )
            bd_tiles[s] = bd_t

        def emit_phase_b(g):
            xwp_t = xwp_pool.tile([128, D], F32, tag="xwp")
            for c4 in range(4):
                nc.tensor.matmul(
                    xwp_t[:], rt_all[:, c4, 128 * g : 128 * (g + 1)],
                    wt_sb[:, c4, :], start=(c4 == 0), stop=(c4 == 3),
                )
            xwb_t = xwb_pool.tile([128, D], BF16, tag="xwb")
            nc.vector.tensor_copy(xwb_t[:], xwp_t[:])

            aggp_t = aggp_pool.tile([NB, D], F32, tag="aggp")
            nc.tensor.matmul(
                aggp_t[:], ones1_sb[:], cb_sb[:], start=True, stop=False,
            )
            nc.tensor.matmul(
                aggp_t[:], bd_tiles[g // 2][:, g % 2, :], xwb_t[:],
                start=False, stop=True,
            )

            # relu + pool-weight mult + free-dim sum fused: one DVE pass
            scr_t = scr_pool.tile([NB, D], F32, tag="scr")
            ns_t = ns_pool.tile([NB, 1], F32, tag="ns")
            nc.vector.scalar_tensor_tensor(
                scr_t[:], aggp_t[:], 0.0, pwb_sb[:],
                mybir.AluOpType.max, mybir.AluOpType.mult, accum_out=ns_t[:],
            )
            nc.tensor.matmul(
                logits_p[:, g : g + 1], hw_sb[:], ns_t[:], start=True, stop=True,
            )

        emit_treduce(0)
        for s in range(1, NCHUNK):
            emit_treduce(s)
            emit_phase_b(2 * (s - 1))
            emit_phase_b(2 * (s - 1) + 1)
        emit_phase_b(2 * (NCHUNK - 1))
        emit_phase_b(2 * (NCHUNK - 1) + 1)

        # softmax over the 7 classes (partition sub-blocks of 7)
        e_t = tail_pool.tile([GPB * C, NG], F32, tag="e")
        nc.scalar.activation(
            e_t[:], logits_p[:], mybir.ActivationFunctionType.Exp, bias=hbb_sb
        )
        sum_p = xwp_pool.tile([GPB, NG], F32, tag="xwp", name="sum_p")
        nc.tensor.matmul(sum_p[:], b7_sb[:], e_t[:], start=True, stop=True)
        ssb_t = tail_pool.tile([GPB, NG], F32, tag="ssb")
        nc.vector.tensor_copy(ssb_t[:], sum_p[:])
        bcast_p = aggp_pool.tile([GPB * C, NG], F32, tag="aggp", name="bcast_p")
        nc.tensor.matmul(bcast_p[:], b7t_sb[:], ssb_t[:], start=True, stop=True)
        rs_t = tail_pool.tile([GPB * C, NG], F32, tag="rs")
        nc.vector.reciprocal(rs_t[:], bcast_p[:])
        res_t = tail_pool.tile([GPB * C, NG], F32, tag="res")
        nc.vector.tensor_mul(res_t[:], e_t[:], rs_t[:])
        nc.sync.dma_start(out.rearrange("(g bi) c -> (bi c) g", bi=GPB), res_t[:])

    nc.compile()
    return nc


_NC_CACHE = None


def _get_nc():
    global _NC_CACHE
    if _NC_CACHE is None:
        _NC_CACHE = _build_nc()
    return _NC_CACHE


def _f32c(x):
    return np.ascontiguousarray(np.asarray(x, dtype=np.float32))


def _prepare_in_maps(real, graph_sigs, W, conv_bias, pool_w, pool_b, head_w, head_b):
    real = _f32c(real)
    graph_sigs = _f32c(graph_sigs)
    W = _f32c(W)
    head_w = _f32c(head_w)

    wt = W.reshape(128, 4, D).astype(NP_BF16)  # wt[p, c, :] = W[4p+c]
    # bf16 blob: ones8 | ones1 | conv_bias
    cbh = np.zeros((128, 392), dtype=NP_BF16)
    for m in range(BPC):
        cbh[16 * m : 16 * (m + 1), m] = np.float32(1.0 / T)
    cbh[0, BPC : BPC + NB] = np.float32(1.0)
    cbh[0, 136 : 136 + D] = _f32c(conv_bias).astype(NP_BF16)
    # f32 blob: pwb | hwblk | hbb | b7 | b7t
    cf = np.zeros((128, 320), dtype=np.float32)
    cf[:, 0:D] = _f32c(pool_w)[None, :]
    # hwblk rows 30k+n -> col k*7+c = head_w[c, n]
    for k in range(GPB):
        cf[N * k : N * (k + 1), D + C * k : D + C * (k + 1)] = head_w.T
    # pool_b shifts every node score by a constant; fold into the head bias
    hb_eff = _f32c(head_b) + np.float32(np.asarray(pool_b)) * head_w.sum(axis=1)
    cf[0 : GPB * C, 284] = np.tile(hb_eff, GPB)
    for k in range(GPB):
        cf[C * k : C * (k + 1), 285 + k] = 1.0
    for k in range(GPB):
        cf[0:GPB, 289 + C * k : 289 + C * (k + 1)] = np.eye(GPB)[:, k : k + 1]

    consts = {"wt": wt, "cbh": cbh, "cf": cf}
    in_maps = []
    for c in range(NCORES):
        s = slice(c * BL, (c + 1) * BL)
        gs8 = graph_sigs[s].reshape(BL * T, NN).astype(NP_BF16)
        rt = real[s].transpose(2, 0, 1).reshape(F_IN, NG, GPB, N)
        rtp = np.zeros((F_IN, NG, GPB, PB), dtype=NP_BF16)
        rtp[:, :, :, 0:N] = rt.astype(NP_BF16)
        in_maps.append(
            {
                "gs8": gs8,
                "rtp": np.ascontiguousarray(rtp.reshape(F_IN, NG * 128)),
                **consts,
            }
        )
    return in_maps


def kernel(real, imag, graph_sigs, W, conv_bias, pool_w, pool_b, head_w, head_b):
    del imag  # unused by the forward pass
    in_maps = _prepare_in_maps(
        real, graph_sigs, W, conv_bias, pool_w, pool_b, head_w, head_b
    )
    nc = _get_nc()
    res = run_bass_kernel_spmd(nc, in_maps, core_ids=list(range(NCORES)))
    return np.concatenate([res.results[c]["out"] for c in range(NCORES)], axis=0)


# revision 23
# speedup vs baseline: 1.6539x; 1.3529x over previous
"""DiGCNNet forward on 8 Trainium2 NeuronCores, data-parallel over batch.

Math (per batch b):
  adj = mean_t graph_sigs[b]                  # [30, 30]
  xw  = real[b] @ W                           # [30, 256]
  agg = adj^T @ xw + conv_bias                # [30, 256]
  h   = relu(agg)
  ns  = h @ pool_w + pool_b                   # [30]
  lg  = ns @ head_w^T + head_b                # [7]
  out = softmax(lg)

Device strategy per core (64 batches, 16 groups of 4, bf16 inputs):
  - gs cast to bf16 on host, stored [4096, 900] (rows = (b, t)).  Loaded in 8
    chunks of 512 rows with partition p <- row 4p+c ("(p c) m"), giving 7.2KB
    contiguous DMA descriptors.  Batch b of a chunk occupies partitions
    [16b, 16b+16) for every c, so a block-structured ones8 [128, 8] matmul
    (accumulated over c=0..3) T-reduces 8 batches -> PSUM [8, 900].
  - adj -> block-diagonal via a DRAM round-trip (SBUF-side DMA APs cannot
    split a free dim across partitions, but DRAM APs have arbitrary strides):
    per group scatter-write adjs[4h:4h+4] into a zero-initialized DRAM
    scratch laid out [128 rows (32k+i), 16 groups, 120 cols (30k+j)], then
    read back per chunk as one [128, 2, 120] tile.  Pad rows stay zero.
  - xw: realT padded to [512, 2048] bf16 on host (group g at cols 128g+32k+n,
    zeros at n=30,31); 4 accumulating matmuls vs W chunks -> PSUM [128, 256].
  - agg: bias matmul (ones x conv_bias) + ONE block-diag matmul
    bd[128, 120]^T @ xwb[128, 256] -> PSUM [120, 256] (rows 30k+j).
  - relu + pool-weight mult + free-dim sum fused in one DVE
    scalar_tensor_tensor (accum_out) -> node scores ns [120, 1].
  - head: [120, 28] matmul -> logits accumulated directly in a PSUM [28, 16]
    tile (one column per group); pool_b/head_b folded into the softmax exp
    bias; softmax tail as 7-block partition sums via tiny matmuls.
"""

from contextlib import ExitStack

import ml_dtypes
import numpy as np

import concourse.bacc as bacc
import concourse.bass as bass
import concourse.tile as tile
from concourse import mybir
from concourse.bass_utils import run_bass_kernel_spmd

F32 = mybir.dt.float32
BF16 = mybir.dt.bfloat16
FP8 = mybir.dt.float8e4
NP_BF16 = ml_dtypes.bfloat16
NP_FP8 = ml_dtypes.float8_e4m3

B, T, N = 512, 64, 30
F_IN, D, C = 512, 256, 7
NCORES = 8
BL = B // NCORES        # 64 batches per core
GPB = 4                 # batches per group
NG = BL // GPB          # 16 groups
NN = N * N              # 900
NB = GPB * N            # 120 packed rows per group
PB = 32                 # padded rows per batch block
NCHUNK = 8              # gs chunks; each = 512 rows = 8 batches = 2 groups
BPC = 8                 # batches per chunk
BDW = NG * NB           # 1920: bd scratch row width


def _build_nc():
    nc = bacc.Bacc(None, target_bir_lowering=False)

    gs8 = nc.dram_tensor("gs8", (BL * T, NN), FP8, kind="ExternalInput")
    ones16 = nc.dram_tensor("ones16", (128, 2, 16), FP8, kind="ExternalInput")
    rtp = nc.dram_tensor("rtp", (F_IN, NG * 128), BF16, kind="ExternalInput")
    wt = nc.dram_tensor("wt", (128, 4, D), BF16, kind="ExternalInput")
    # consolidated constants: one bf16 blob, one f32 blob (single DMA each)
    # cbh[:, 0:8] = ones8; cbh[0, 8:136] = ones1; cbh[0, 136:392] = conv_bias
    cbh = nc.dram_tensor("cbh", (128, 392), BF16, kind="ExternalInput")
    # cf[:, 0:256] = pwb; cf[0:120, 256:284] = hwblk; cf[0:28, 284] = hbb;
    # cf[0:28, 285:289] = b7; cf[0:4, 289:317] = b7t
    cf = nc.dram_tensor("cf", (128, 320), F32, kind="ExternalInput")
    # block-diag scratch: element (32k+i)*1920 + g*120 + 30k+j
    bdram = nc.dram_tensor("bdram", (128, BDW), BF16, kind="Internal")
    out = nc.dram_tensor("out", (BL, C), F32, kind="ExternalOutput")

    with tile.TileContext(nc) as tc, ExitStack() as ctx:
        consts = ctx.enter_context(tc.tile_pool(name="consts", bufs=1))
        gs_pool = ctx.enter_context(tc.tile_pool(name="gsp", bufs=NCHUNK))
        adjs_pool = ctx.enter_context(tc.tile_pool(name="adjs", bufs=4))
        bd_pool = ctx.enter_context(tc.tile_pool(name="bd", bufs=3))
        xwb_pool = ctx.enter_context(tc.tile_pool(name="xwb", bufs=2))
        scr_pool = ctx.enter_context(tc.tile_pool(name="scr", bufs=2))
        ns_pool = ctx.enter_context(tc.tile_pool(name="ns", bufs=2))
        tail_pool = ctx.enter_context(tc.tile_pool(name="tail", bufs=1))
        adjp_pool = ctx.enter_context(
            tc.tile_pool(name="adjp", bufs=2, space=bass.MemorySpace.PSUM)
        )
        xwp_pool = ctx.enter_context(
            tc.tile_pool(name="xwp", bufs=2, space=bass.MemorySpace.PSUM)
        )
        aggp_pool = ctx.enter_context(
            tc.tile_pool(name="aggp", bufs=1, space=bass.MemorySpace.PSUM)
        )
        lgp_pool = ctx.enter_context(
            tc.tile_pool(name="lgp", bufs=1, space=bass.MemorySpace.PSUM)
        )

        # consts + weights on the scalar queue (gs stream owns sync)
        ones16_sb = consts.tile([128, 2, 16], FP8, tag="ones16")
        nc.scalar.dma_start(ones16_sb[:], ones16[:])
        cbh_sb = consts.tile([128, 392], BF16, tag="cbh")
        nc.scalar.dma_start(cbh_sb[:], cbh[:])
        cf_sb = consts.tile([128, 320], F32, tag="cf")
        nc.scalar.dma_start(cf_sb[:], cf[:])
        ones1_sb = cbh_sb[0:1, BPC : BPC + NB]
        cb_sb = cbh_sb[0:1, 136 : 136 + D]
        pwb_sb = cf_sb[0:NB, 0:D]
        hw_sb = cf_sb[0:NB, D : D + GPB * C]
        hbb_sb = cf_sb[0 : GPB * C, 284:285]
        b7_sb = cf_sb[0 : GPB * C, 285:289]
        b7t_sb = cf_sb[0:GPB, 289:317]

        wt_sb = consts.tile([128, 4, D], BF16, tag="wt")
        nc.scalar.dma_start(wt_sb[:], wt[:])
        rt_all = consts.tile([128, 4, NG * 128], BF16, tag="rt_all")
        for h4 in range(4):
            cs = h4 * 512
            nc.scalar.dma_start(
                rt_all[:, :, cs : cs + 512],
                rtp[:, cs : cs + 512].rearrange("(p c) m -> p c m", c=4),
            )

        # zero-init the bd scratch (one contiguous write)
        zt = consts.tile([128, BDW], BF16, tag="zt")
        nc.vector.memset(zt[:], 0.0)
        nc.scalar.dma_start(bdram[:], zt[:])

        # gs stream: 8 chunks of [128, 4, 900] fp8 on the sync queue
        gs_tiles = []
        for s in range(NCHUNK):
            gt = gs_pool.tile([128, 4, NN], FP8, tag="gt", name=f"gt{s}")
            nc.sync.dma_start(
                gt[:],
                gs8[512 * s : 512 * (s + 1)].rearrange("(p c) m -> p c m", c=4),
            )
            gs_tiles.append(gt)

        logits_p = lgp_pool.tile([GPB * C, NG], F32, tag="logits")
        bd_tiles = {}

        def emit_treduce(s):
            # fp8 DoubleRow: K=256 per matmul (two c-columns), each batch
            # duplicated on 2 out rows (M=16; M=8 trips lw_dual_fp8 ISA check)
            adjp_t = adjp_pool.tile([2 * BPC, NN], F32, tag="adjp")
            for h in range(2):
                for lo, hi in ((0, 512), (512, NN)):
                    nc.tensor.matmul(
                        adjp_t[:, lo:hi], ones16_sb[:],
                        gs_tiles[s][:, 2 * h : 2 * h + 2, lo:hi],
                        start=(h == 0), stop=(h == 1),
                        perf_mode=mybir.MatmulPerfMode.DoubleRow,
                    )
            adjs_t = adjs_pool.tile([2 * BPC, NN], BF16, tag="adjs")
            nc.vector.tensor_copy(adjs_t[:, 0:450], adjp_t[:, 0:450])
            nc.scalar.copy(adjs_t[:, 450:NN], adjp_t[:, 450:NN])
            # scatter-write the two groups' diag blocks into the DRAM scratch
            adjs_ev = adjs_t[:].rearrange("(b r) m -> b r m", r=2)
            for half, eng in ((0, nc.scalar), (1, nc.gpsimd)):
                g = 2 * s + half
                wdst = bass.AP(
                    bdram, NB * g, [[PB * BDW + N, GPB], [BDW, N], [1, N]]
                )
                eng.dma_start(wdst, adjs_ev[4 * half : 4 * half + 4, 0, :])
            # read back both groups as one [128, 2, 120] block-diag tile
            bd_t = bd_pool.tile([128, 2, NB], BF16, tag="bd", name=f"bd{s}")
            nc.gpsimd.dma_start(
                bd_t[:],
                bdram[:, 2 * NB * s : 2 * NB * (s + 1)].rearrange(
                    "p (h m) -> p h m", h=2
                ),
            )
            bd_tiles[s] = bd_t

        def emit_phase_b(g):
            xwp_t = xwp_pool.tile([128, D], F32, tag="xwp")
            for c4 in range(4):
                nc.tensor.matmul(
                    xwp_t[:], rt_all[:, c4, 128 * g : 128 * (g + 1)],
                    wt_sb[:, c4, :], start=(c4 == 0), stop=(c4 == 3),
                )
            xwb_t = xwb_pool.tile([128, D], BF16, tag="xwb")
            nc.vector.tensor_copy(xwb_t[:], xwp_t[:])

            aggp_t = aggp_pool.tile([NB, D], F32, tag="aggp")
            nc.tensor.matmul(
                aggp_t[:], ones1_sb[:], cb_sb[:], start=True, stop=False,
            )
            nc.tensor.matmul(
                aggp_t[:], bd_tiles[g // 2][:, g % 2, :], xwb_t[:],
                start=False, stop=True,
            )

            # relu + pool-weight mult + free-dim sum fused: one DVE pass
            scr_t = scr_pool.tile([NB, D], F32, tag="scr")
            ns_t = ns_pool.tile([NB, 1], F32, tag="ns")
            nc.vector.scalar_tensor_tensor(
                scr_t[:], aggp_t[:], 0.0, pwb_sb[:],
                mybir.AluOpType.max, mybir.AluOpType.mult, accum_out=ns_t[:],
            )
            nc.tensor.matmul(
                logits_p[:, g : g + 1], hw_sb[:], ns_t[:], start=True, stop=True,
            )

        emit_treduce(0)
        for s in range(1, NCHUNK):
            emit_treduce(s)
            emit_phase_b(2 * (s - 1))
            emit_phase_b(2 * (s - 1) + 1)
        emit_phase_b(2 * (NCHUNK - 1))
        emit_phase_b(2 * (NCHUNK - 1) + 1)

        # softmax over the 7 classes (partition sub-blocks of 7)
        e_t = tail_pool.tile([GPB * C, NG], F32, tag="e")
        nc.scalar.activation(
            e_t[:], logits_p[:], mybir.ActivationFunctionType.Exp, bias=hbb_sb
        )
        sum_p = xwp_pool.tile([GPB, NG], F32, tag="xwp", name="sum_p")
        nc.tensor.matmul(sum_p[:], b7_sb[:], e_t[:], start=True, stop=True)
        ssb_t = tail_pool.tile([GPB, NG], F32, tag="ssb")
        nc.vector.tensor_copy(ssb_t[:], sum_p[:])
        bcast_p = aggp_pool.tile([GPB * C, NG], F32, tag="aggp", name="bcast_p")
        nc.tensor.matmul(bcast_p[:], b7t_sb[:], ssb_t[:], start=True, stop=True)
        rs_t = tail_pool.tile([GPB * C, NG], F32, tag="rs")
        nc.vector.reciprocal(rs_t[:], bcast_p[:])
        res_t = tail_pool.tile([GPB * C, NG], F32, tag="res")
        nc.vector.tensor_mul(res_t[:], e_t[:], rs_t[:])
        nc.sync.dma_start(out.rearrange("(g bi) c -> (bi c) g", bi=GPB), res_t[:])

    nc.compile()
    return nc


_NC_CACHE = None


def _get_nc():
    global _NC_CACHE
    if _NC_CACHE is None:
        _NC_CACHE = _build_nc()
    return _NC_CACHE


def _f32c(x):
    return np.ascontiguousarray(np.asarray(x, dtype=np.float32))


def _prepare_in_maps(real, graph_sigs, W, conv_bias, pool_w, pool_b, head_w, head_b):
    real = _f32c(real)
    graph_sigs = _f32c(graph_sigs)
    W = _f32c(W)
    head_w = _f32c(head_w)

    wt = W.reshape(128, 4, D).astype(NP_BF16)  # wt[p, c, :] = W[4p+c]
    # fp8 DoubleRow T-reduce weights: batch m//2 at partitions [16b, 16b+16)
    ones16 = np.zeros((128, 2, 16), dtype=np.float32)
    for m in range(16):
        b = m // 2
        ones16[16 * b : 16 * (b + 1), :, m] = np.float32(1.0 / T)
    ones16 = ones16.astype(NP_FP8)
    # bf16 blob: (unused) | ones1 | conv_bias
    cbh = np.zeros((128, 392), dtype=NP_BF16)
    cbh[0, BPC : BPC + NB] = np.float32(1.0)
    cbh[0, 136 : 136 + D] = _f32c(conv_bias).astype(NP_BF16)
    # f32 blob: pwb | hwblk | hbb | b7 | b7t
    cf = np.zeros((128, 320), dtype=np.float32)
    cf[:, 0:D] = _f32c(pool_w)[None, :]
    # hwblk rows 30k+n -> col k*7+c = head_w[c, n]
    for k in range(GPB):
        cf[N * k : N * (k + 1), D + C * k : D + C * (k + 1)] = head_w.T
    # pool_b shifts every node score by a constant; fold into the head bias
    hb_eff = _f32c(head_b) + np.float32(np.asarray(pool_b)) * head_w.sum(axis=1)
    cf[0 : GPB * C, 284] = np.tile(hb_eff, GPB)
    for k in range(GPB):
        cf[C * k : C * (k + 1), 285 + k] = 1.0
    for k in range(GPB):
        cf[0:GPB, 289 + C * k : 289 + C * (k + 1)] = np.eye(GPB)[:, k : k + 1]

    consts = {"wt": wt, "cbh": cbh, "cf": cf, "ones16": ones16}
    in_maps = []
    for c in range(NCORES):
        s = slice(c * BL, (c + 1) * BL)
        gs8 = graph_sigs[s].reshape(BL * T, NN).astype(NP_FP8)
        rt = real[s].transpose(2, 0, 1).reshape(F_IN, NG, GPB, N)
        rtp = np.zeros((F_IN, NG, GPB, PB), dtype=NP_BF16)
        rtp[:, :, :, 0:N] = rt.astype(NP_BF16)
        in_maps.append(
            {
                "gs8": gs8,
                "rtp": np.ascontiguousarray(rtp.reshape(F_IN, NG * 128)),
                **consts,
            }
        )
    return in_maps


def kernel(real, imag, graph_sigs, W, conv_bias, pool_w, pool_b, head_w, head_b):
    del imag  # unused by the forward pass
    in_maps = _prepare_in_maps(
        real, graph_sigs, W, conv_bias, pool_w, pool_b, head_w, head_b
    )
    nc = _get_nc()
    res = run_bass_kernel_spmd(nc, in_maps, core_ids=list(range(NCORES)))
    return np.concatenate([res.results[c]["out"] for c in range(NCORES)], axis=0)


# revision 27
# speedup vs baseline: 1.6743x; 1.0123x over previous
"""DiGCNNet forward on 8 Trainium2 NeuronCores, data-parallel over batch.

Math (per batch b):
  adj = mean_t graph_sigs[b]                  # [30, 30]
  xw  = real[b] @ W                           # [30, 256]
  agg = adj^T @ xw + conv_bias                # [30, 256]
  h   = relu(agg)
  ns  = h @ pool_w + pool_b                   # [30]
  lg  = ns @ head_w^T + head_b                # [7]
  out = softmax(lg)

Device strategy per core (64 batches, 16 groups of 4, bf16 inputs):
  - gs cast to bf16 on host, stored [4096, 900] (rows = (b, t)).  Loaded in 8
    chunks of 512 rows with partition p <- row 4p+c ("(p c) m"), giving 7.2KB
    contiguous DMA descriptors.  Batch b of a chunk occupies partitions
    [16b, 16b+16) for every c, so a block-structured ones8 [128, 8] matmul
    (accumulated over c=0..3) T-reduces 8 batches -> PSUM [8, 900].
  - adj -> block-diagonal via a DRAM round-trip (SBUF-side DMA APs cannot
    split a free dim across partitions, but DRAM APs have arbitrary strides):
    per group scatter-write adjs[4h:4h+4] into a zero-initialized DRAM
    scratch laid out [128 rows (32k+i), 16 groups, 120 cols (30k+j)], then
    read back per chunk as one [128, 2, 120] tile.  Pad rows stay zero.
  - xw: realT padded to [512, 2048] bf16 on host (group g at cols 128g+32k+n,
    zeros at n=30,31); 4 accumulating matmuls vs W chunks -> PSUM [128, 256].
  - agg: bias matmul (ones x conv_bias) + ONE block-diag matmul
    bd[128, 120]^T @ xwb[128, 256] -> PSUM [120, 256] (rows 30k+j).
  - relu + pool-weight mult + free-dim sum fused in one DVE
    scalar_tensor_tensor (accum_out) -> node scores ns [120, 1].
  - head: [120, 28] matmul -> logits accumulated directly in a PSUM [28, 16]
    tile (one column per group); pool_b/head_b folded into the softmax exp
    bias; softmax tail as 7-block partition sums via tiny matmuls.
"""

from contextlib import ExitStack

import ml_dtypes
import numpy as np

import concourse.bacc as bacc
import concourse.bass as bass
import concourse.tile as tile
from concourse import mybir
from concourse.bass_utils import run_bass_kernel_spmd

F32 = mybir.dt.float32
BF16 = mybir.dt.bfloat16
FP8 = mybir.dt.float8e4
NP_BF16 = ml_dtypes.bfloat16
NP_FP8 = ml_dtypes.float8_e4m3

B, T, N = 512, 64, 30
F_IN, D, C = 512, 256, 7
NCORES = 8
BL = B // NCORES        # 64 batches per core
GPB = 4                 # batches per group
NG = BL // GPB          # 16 groups
NN = N * N              # 900
NB = GPB * N            # 120 packed rows per group
PB = 32                 # padded rows per batch block
NCHUNK = 8              # gs chunks; each = 512 rows = 8 batches = 2 groups
BPC = 8                 # batches per chunk
BDW = NG * NB           # 1920: bd scratch row width


def _build_nc():
    nc = bacc.Bacc(None, target_bir_lowering=False)

    gs8 = nc.dram_tensor("gs8", (BL * T, NN), FP8, kind="ExternalInput")
    ones16 = nc.dram_tensor("ones16", (128, 2, 16), FP8, kind="ExternalInput")
    rtp = nc.dram_tensor("rtp", (F_IN, NG * 128), BF16, kind="ExternalInput")
    wt = nc.dram_tensor("wt", (128, 4, D), BF16, kind="ExternalInput")
    # consolidated constants: one bf16 blob, one f32 blob (single DMA each)
    # cbh[0, 8:136] = ones1; cbh[0, 136:648] = conv_bias x2
    cbh = nc.dram_tensor("cbh", (128, 648), BF16, kind="ExternalInput")
    # cf[:, 0:256] = pwb; cf[0:120, 256:284] = hwblk; cf[0:28, 284] = hbb;
    # cf[0:28, 285:289] = b7; cf[0:4, 289:317] = b7t
    cf = nc.dram_tensor("cf", (128, 320), F32, kind="ExternalInput")
    # block-diag scratch: element (32k+i)*1920 + g*120 + 30k+j
    bdram = nc.dram_tensor("bdram", (128, BDW), BF16, kind="Internal")
    out = nc.dram_tensor("out", (BL, C), F32, kind="ExternalOutput")

    with tile.TileContext(nc) as tc, ExitStack() as ctx:
        consts = ctx.enter_context(tc.tile_pool(name="consts", bufs=1))
        gs_pool = ctx.enter_context(tc.tile_pool(name="gsp", bufs=NCHUNK))
        adjs_pool = ctx.enter_context(tc.tile_pool(name="adjs", bufs=4))
        bd_pool = ctx.enter_context(tc.tile_pool(name="bd", bufs=3))
        xwb_pool = ctx.enter_context(tc.tile_pool(name="xwb", bufs=2))
        scr_pool = ctx.enter_context(tc.tile_pool(name="scr", bufs=2))
        ns_pool = ctx.enter_context(tc.tile_pool(name="ns", bufs=2))
        tail_pool = ctx.enter_context(tc.tile_pool(name="tail", bufs=1))
        adjp_pool = ctx.enter_context(
            tc.tile_pool(name="adjp", bufs=2, space=bass.MemorySpace.PSUM)
        )
        xwp_pool = ctx.enter_context(
            tc.tile_pool(name="xwp", bufs=2, space=bass.MemorySpace.PSUM)
        )
        aggp_pool = ctx.enter_context(
            tc.tile_pool(name="aggp", bufs=1, space=bass.MemorySpace.PSUM)
        )
        lgp_pool = ctx.enter_context(
            tc.tile_pool(name="lgp", bufs=1, space=bass.MemorySpace.PSUM)
        )

        # consts + weights on the scalar queue (gs stream owns sync)
        ones16_sb = consts.tile([128, 2, 16], FP8, tag="ones16")
        nc.scalar.dma_start(ones16_sb[:], ones16[:])
        cbh_sb = consts.tile([128, 648], BF16, tag="cbh")
        nc.scalar.dma_start(cbh_sb[:], cbh[:])
        cf_sb = consts.tile([128, 320], F32, tag="cf")
        nc.scalar.dma_start(cf_sb[:], cf[:])
        ones1_sb = cbh_sb[0:1, BPC : BPC + NB]
        cb2_sb = cbh_sb[0:1, 136 : 136 + 2 * D]
        pwb_sb = cf_sb[0:NB, 0:D]
        hw_sb = cf_sb[0:NB, D : D + GPB * C]
        hbb_sb = cf_sb[0 : GPB * C, 284:285]
        b7_sb = cf_sb[0 : GPB * C, 285:289]
        b7t_sb = cf_sb[0:GPB, 289:317]

        wt_sb = consts.tile([128, 4, D], BF16, tag="wt")
        nc.scalar.dma_start(wt_sb[:], wt[:])
        rt_all = consts.tile([128, 4, NG * 128], BF16, tag="rt_all")
        for h4 in range(2):
            cs = h4 * 512
            nc.scalar.dma_start(
                rt_all[:, :, cs : cs + 512],
                rtp[:, cs : cs + 512].rearrange("(p c) m -> p c m", c=4),
            )

        # zero-init the bd scratch (one contiguous write)
        zt = consts.tile([128, BDW], BF16, tag="zt")
        nc.vector.memset(zt[:], 0.0)
        nc.scalar.dma_start(bdram[:], zt[:])

        # gs stream: 8 chunks of [128, 4, 900] fp8 on the sync queue;
        # rt halves 2,3 ride behind it (needed only by groups 8+)
        gs_tiles = []
        for s in range(NCHUNK):
            gt = gs_pool.tile([128, 4, NN], FP8, tag="gt", name=f"gt{s}")
            nc.sync.dma_start(
                gt[:],
                gs8[512 * s : 512 * (s + 1)].rearrange("(p c) m -> p c m", c=4),
            )
            gs_tiles.append(gt)
        for h4 in range(2, 4):
            cs = h4 * 512
            nc.sync.dma_start(
                rt_all[:, :, cs : cs + 512],
                rtp[:, cs : cs + 512].rearrange("(p c) m -> p c m", c=4),
            )

        logits_p = lgp_pool.tile([GPB * C, NG], F32, tag="logits")
        bd_tiles = {}

        def emit_treduce(s):
            # fp8 DoubleRow: K=256 per matmul (two c-columns), each batch
            # duplicated on 2 out rows (M=16; M=8 trips lw_dual_fp8 ISA check)
            adjp_t = adjp_pool.tile([2 * BPC, NN], F32, tag="adjp")
            for h in range(2):
                for lo, hi in ((0, 512), (512, NN)):
                    nc.tensor.matmul(
                        adjp_t[:, lo:hi], ones16_sb[:],
                        gs_tiles[s][:, 2 * h : 2 * h + 2, lo:hi],
                        start=(h == 0), stop=(h == 1),
                        perf_mode=mybir.MatmulPerfMode.DoubleRow,
                    )
            adjs_t = adjs_pool.tile([2 * BPC, NN], BF16, tag="adjs")
            nc.vector.tensor_copy(adjs_t[:, 0:300], adjp_t[:, 0:300])
            nc.scalar.copy(adjs_t[:, 300:NN], adjp_t[:, 300:NN])
            # scatter-write the two groups' diag blocks into the DRAM scratch
            adjs_ev = adjs_t[:].rearrange("(b r) m -> b r m", r=2)
            for half in range(2):
                g = 2 * s + half
                wdst = bass.AP(
                    bdram, NB * g, [[PB * BDW + N, GPB], [BDW, N], [1, N]]
                )
                nc.gpsimd.dma_start(wdst, adjs_ev[4 * half : 4 * half + 4, 0, :])
            # read back both groups as one [128, 2, 120] block-diag tile
            bd_t = bd_pool.tile([128, 2, NB], BF16, tag="bd", name=f"bd{s}")
            nc.scalar.dma_start(
                bd_t[:],
                bdram[:, 2 * NB * s : 2 * NB * (s + 1)].rearrange(
                    "p (h m) -> p h m", h=2
                ),
            )
            bd_tiles[s] = bd_t

        def emit_phase_pair(s):
            # groups 2s, 2s+1 share one PSUM pair tile and one head matmul
            aggp_t = aggp_pool.tile([NB, 2, D], F32, tag="aggp")
            nc.tensor.matmul(
                aggp_t[:], ones1_sb[:], cb2_sb[:],
                start=True, stop=False, skip_group_check=True,
            )
            xwp_t = xwp_pool.tile([128, 2, D], F32, tag="xwp")
            ns2_t = ns_pool.tile([NB, 2], F32, tag="ns")
            for h in range(2):
                g = 2 * s + h
                for c4 in range(4):
                    nc.tensor.matmul(
                        xwp_t[:, h, :], rt_all[:, c4, 128 * g : 128 * (g + 1)],
                        wt_sb[:, c4, :], start=(c4 == 0), stop=(c4 == 3),
                    )
            xwb_t = xwb_pool.tile([128, 2, D], BF16, tag="xwb")
            nc.vector.tensor_copy(xwb_t[:], xwp_t[:])
            for h in range(2):
                nc.tensor.matmul(
                    aggp_t[:, h, :], bd_tiles[s][:, h, :], xwb_t[:, h, :],
                    start=False, stop=(h == 1), skip_group_check=True,
                )
            # relu + pool-weight mult + free-dim sum fused: one DVE pass each
            for h in range(2):
                scr_t = scr_pool.tile([NB, D], F32, tag="scr")
                nc.vector.scalar_tensor_tensor(
                    scr_t[:], aggp_t[:, h, :], 0.0, pwb_sb[:],
                    mybir.AluOpType.max, mybir.AluOpType.mult,
                    accum_out=ns2_t[:, h : h + 1],
                )
            nc.tensor.matmul(
                logits_p[:, 2 * s : 2 * s + 2], hw_sb[:], ns2_t[:],
                start=True, stop=True,
            )

        emit_treduce(0)
        for s in range(1, NCHUNK):
            emit_treduce(s)
            emit_phase_pair(s - 1)
        emit_phase_pair(NCHUNK - 1)

        # softmax over the 7 classes (partition sub-blocks of 7)
        e_t = tail_pool.tile([GPB * C, NG], F32, tag="e")
        nc.scalar.activation(
            e_t[:], logits_p[:], mybir.ActivationFunctionType.Exp, bias=hbb_sb
        )
        sum_p = xwp_pool.tile([GPB, NG], F32, tag="xwp", name="sum_p")
        nc.tensor.matmul(sum_p[:], b7_sb[:], e_t[:], start=True, stop=True)
        ssb_t = tail_pool.tile([GPB, NG], F32, tag="ssb")
        nc.vector.tensor_copy(ssb_t[:], sum_p[:])
        bcast_p = aggp_pool.tile([GPB * C, NG], F32, tag="aggp", name="bcast_p")
        nc.tensor.matmul(bcast_p[:], b7t_sb[:], ssb_t[:], start=True, stop=True)
        rs_t = tail_pool.tile([GPB * C, NG], F32, tag="rs")
        nc.vector.reciprocal(rs_t[:], bcast_p[:])
        res_t = tail_pool.tile([GPB * C, NG], F32, tag="res")
        nc.vector.tensor_mul(res_t[:], e_t[:], rs_t[:])
        nc.sync.dma_start(out.rearrange("(g bi) c -> (bi c) g", bi=GPB), res_t[:])

    nc.compile()
    return nc


_NC_CACHE = None


def _get_nc():
    global _NC_CACHE
    if _NC_CACHE is None:
        _NC_CACHE = _build_nc()
    return _NC_CACHE


def _f32c(x):
    return np.ascontiguousarray(np.asarray(x, dtype=np.float32))


def _prepare_in_maps(real, graph_sigs, W, conv_bias, pool_w, pool_b, head_w, head_b):
    real = _f32c(real)
    graph_sigs = _f32c(graph_sigs)
    W = _f32c(W)
    head_w = _f32c(head_w)

    wt = W.reshape(128, 4, D).astype(NP_BF16)  # wt[p, c, :] = W[4p+c]
    # fp8 DoubleRow T-reduce weights: batch m//2 at partitions [16b, 16b+16)
    ones16 = np.zeros((128, 2, 16), dtype=np.float32)
    for m in range(16):
        b = m // 2
        ones16[16 * b : 16 * (b + 1), :, m] = np.float32(1.0 / T)
    ones16 = ones16.astype(NP_FP8)
    # bf16 blob: ones1 | conv_bias x2
    cbh = np.zeros((128, 648), dtype=NP_BF16)
    cbh[0, BPC : BPC + NB] = np.float32(1.0)
    cbh[0, 136 : 136 + D] = _f32c(conv_bias).astype(NP_BF16)
    cbh[0, 136 + D : 136 + 2 * D] = _f32c(conv_bias).astype(NP_BF16)
    # f32 blob: pwb | hwblk | hbb | b7 | b7t
    cf = np.zeros((128, 320), dtype=np.float32)
    cf[:, 0:D] = _f32c(pool_w)[None, :]
    # hwblk rows 30k+n -> col k*7+c = head_w[c, n]
    for k in range(GPB):
        cf[N * k : N * (k + 1), D + C * k : D + C * (k + 1)] = head_w.T
    # pool_b shifts every node score by a constant; fold into the head bias
    hb_eff = _f32c(head_b) + np.float32(np.asarray(pool_b)) * head_w.sum(axis=1)
    cf[0 : GPB * C, 284] = np.tile(hb_eff, GPB)
    for k in range(GPB):
        cf[C * k : C * (k + 1), 285 + k] = 1.0
    for k in range(GPB):
        cf[0:GPB, 289 + C * k : 289 + C * (k + 1)] = np.eye(GPB)[:, k : k + 1]

    consts = {"wt": wt, "cbh": cbh, "cf": cf, "ones16": ones16}
    in_maps = []
    for c in range(NCORES):
        s = slice(c * BL, (c + 1) * BL)
        gs8 = graph_sigs[s].reshape(BL * T, NN).astype(NP_FP8)
        rt = real[s].transpose(2, 0, 1).reshape(F_IN, NG, GPB, N)
        rtp = np.zeros((F_IN, NG, GPB, PB), dtype=NP_BF16)
        rtp[:, :, :, 0:N] = rt.astype(NP_BF16)
        in_maps.append(
            {
                "gs8": gs8,
                "rtp": np.ascontiguousarray(rtp.reshape(F_IN, NG * 128)),
                **consts,
            }
        )
    return in_maps


def kernel(real, imag, graph_sigs, W, conv_bias, pool_w, pool_b, head_w, head_b):
    del imag  # unused by the forward pass
    in_maps = _prepare_in_maps(
        real, graph_sigs, W, conv_bias, pool_w, pool_b, head_w, head_b
    )
    nc = _get_nc()
    res = run_bass_kernel_spmd(nc, in_maps, core_ids=list(range(NCORES)))
    return np.concatenate([res.results[c]["out"] for c in range(NCORES)], axis=0)
